# revision 10
# baseline (speedup 1.0000x reference)
"""Multi-head self-attention Trainium2 Bass kernel, v3.

Sharding (unchanged from v2): core = (batch b, head-group g of 8 heads);
the host sums the two head-groups' partial projections.

v3 is a ground-up reschedule driven by HW microbenchmarks (the cost
model badly underestimates fine-grained matmul overhead on this part):

- The softmax denominator is folded into the AV matmul: v tiles carry a
  ones column per head (stride 65), so each AV group is ONE matmul of
  N=65 instead of MM(64) + MM(1) + an extra weight reload.  The N=1
  denominator matmuls were ~500ns/group on HW (PSUM zero-region cost);
  the merged form measures ~25ns/group.
- One exp ACT per (pair, win, kc) covering both heads ([128,1024] PSUM
  -> SBUF bf16, ~1.07us measured).  256 ACTs ~= 275us is the kernel's
  hard floor (33.5M exps/core at 1 elem/lane/cycle @1.2GHz); the whole
  schedule exists to keep the ACT queue full.
- Scores ping-pong between two 2-bank PSUM regions so scores(i+1) runs
  while ACT(i) drains; AV(i) is emitted AFTER scores(i+1) so the
  in-order PE queue never parks ACT's inputs behind an et-wait.
- Dense work (QKV projection, PE transposes, output projection) is a
  deque of ~1-2us filler items popped into the per-iteration PE slack.
"""

from collections import deque
from contextlib import ExitStack

import bass_rust as _br
import numpy as np
import ml_dtypes

import concourse.bass as bass
import concourse.bacc as bacc
import concourse.tile as tile
from concourse import mybir
from concourse.bass_utils import run_bass_kernel_spmd

N_CORES = 8
C = 1024           # embed dim
H = 16             # total heads
HD = 64            # head dim
HPC = 8            # heads per core
CG = HPC * HD      # 512: per-core q/k/v feature width
NPAIR = HPC // 2   # 4 head pairs
QW = 512           # query window

F32 = mybir.dt.float32
BF16 = mybir.dt.bfloat16


def _order_after(inst, first):
    deps = _br.InstructionNameOrderedSet()
    deps.add(first.ins.name)
    inst.ins.add_nosync_dependencies_from(deps)


def _sync_after(inst, first):
    deps = _br.InstructionNameOrderedSet()
    deps.add(first.ins.name)
    inst.ins.add_sync_dependencies_from(deps)


def _body(tc, T, x_t, w_qk, b_qk, w_v, w_p, b_out, id_bf, out_t):
    nc = tc.nc
    KC = C // 128            # 8 contraction chunks over C
    FC = 2 * CG // 128       # 8 q||k feature chunks
    TC1 = T // 128           # key chunks of 128
    NW = T // QW             # query windows per pair
    NQ = QW // 128           # 4 query chunks per window
    OCC = C // 128           # 8 output channel chunks
    PCH = CG // 128          # 4 proj contraction chunks
    Exp = mybir.ActivationFunctionType.Exp
    Mult = mybir.AluOpType.mult

    with ExitStack() as stack:
        constp = stack.enter_context(tc.tile_pool(name="const", bufs=1))
        pers = stack.enter_context(tc.tile_pool(name="persist", bufs=1))
        ep = stack.enter_context(tc.tile_pool(name="e", bufs=6))
        recp = stack.enter_context(tc.tile_pool(name="rec", bufs=2))
        osbp = stack.enter_context(tc.tile_pool(name="osb", bufs=4))
        psp = stack.enter_context(tc.tile_pool(name="ps", bufs=1,
                                               space="PSUM"))

        # ---- constants / small inputs first in DMA queue order
        bqk_sb = constp.tile([128, FC], F32, tag="bqk")
        nc.sync.dma_start(bqk_sb[:], b_qk[:])
        bout_sb = constp.tile([128, OCC], F32, tag="bout")
        nc.sync.dma_start(bout_sb[:], b_out[:])
        id_sb = constp.tile([128, 128], BF16, tag="id")

        # w_qk: all 8 feature chunks in one tile, pair0's chunks (fc 0 and
        # 4) DMA'd first so the upfront QK(pair0) can start immediately.
        wqk_all = pers.tile([128, FC * KC * 128], BF16, tag="wqkall")

        def wqk_t(fc, kc):
            return wqk_all[:, (fc * KC + kc) * 128:(fc * KC + kc) * 128 + 128]

        def load_wqk(fc):
            nc.sync.dma_start(
                wqk_all[:, fc * KC * 128:(fc + 1) * KC * 128]
                .rearrange("p (k c) -> p k c", c=128),
                w_qk[:, fc * 128:(fc + 1) * 128]
                .rearrange("(k p) c -> p k c", p=128))

        load_wqk(0)
        load_wqk(NPAIR)

        # activations [C, T] bf16: first halves -> w_v -> second halves
        TH = T // 2
        xt = []
        for i in range(KC):
            t = pers.tile([128, T], BF16, tag=f"xt{i}")
            nc.sync.dma_start(t[:, 0:TH], x_t[i * 128:(i + 1) * 128, 0:TH])
            xt.append(t)
        wv_all = pers.tile([128, KC * CG], BF16, tag="wvall")
        nc.sync.dma_start(
            wv_all.rearrange("p (k c) -> p k c", c=CG),
            w_v.rearrange("(k p) c -> p k c", p=128))
        wv = [wv_all[:, kc * CG:(kc + 1) * CG] for kc in range(KC)]
        nc.sync.dma_start(id_sb[:], id_bf[:])
        for fc in range(FC):
            if fc not in (0, NPAIR):
                load_wqk(fc)
        for i in range(KC):
            nc.sync.dma_start(xt[i][:, TH:T], x_t[i * 128:(i + 1) * 128, TH:T])
        wp_all = pers.tile([128, PCH * C], BF16, tag="wpall")
        nc.sync.dma_start(
            wp_all.rearrange("p (f c) -> p f c", c=C),
            w_p.rearrange("(f p) c -> p f c", p=128))
        wp = [wp_all[:, fcp * C:(fcp + 1) * C] for fcp in range(PCH)]

        # ---- persistent intermediates
        qkt = [pers.tile([128, T], BF16, tag=f"qkt{i}", name=f"qkt{i}")
               for i in range(FC)]
        # v2[tokc]: [128 tok, 8 heads x 65]; col h*65+64 stays 1.0 (the
        # memset) so the AV matmul's 65th output column is the softmax
        # denominator.
        v2 = [pers.tile([128, HPC * 65], BF16, tag=f"v2_{i}", name=f"v2_{i}")
              for i in range(TC1)]
        for t in v2:
            nc.vector.memset(t[:], 1.0)
        # ysb[pair][s]: [128 tok-part, (T//512) * 4qi * 64] normalized y
        ysb = [[pers.tile([128, NW * NQ * 64], BF16, tag=f"y{p}_{s}",
                           name=f"y{p}_{s}") for s in range(2)]
               for p in range(NPAIR)]
        # yt[pair]: [128 feat, T] transposed for the projection
        yt = [pers.tile([128, T], BF16, tag=f"yt{i}", name=f"yt{i}")
              for i in range(NPAIR)]

        # ---- PSUM: 2 score regions (2 banks each) + 2 ups + 2 dense
        sreg = [psp.tile([128, 1024], F32, tag=f"sreg{i}", name=f"sreg{i}")
                for i in range(2)]
        ups = [psp.tile([128, 512], F32, tag=f"ups{s}", name=f"ups{s}")
               for s in range(2)]
        # d0: dense accumulation chains (qk/v/proj).  d1: PE transposes
        # ONLY — bitcast views do not participate in automatic dependency
        # tracking, so the transpose bank is dedicated and manually chained
        # with nosync edges (transposes -> copy -> next item's transposes).
        dbank = [psp.tile([128, 512], F32, tag=f"d{i}", name=f"d{i}")
                 for i in range(2)]
        last_tp_copy = [None]
        av_last = {}         # s -> last AV matmul of current window
        norm_mul = {}        # (pair, win, s) -> normalize mult instruction

        # ---------- dense work items ----------
        def qk_item(fc, t4w, quarter):
            ps = dbank[0]
            for kc in range(quarter * 2, quarter * 2 + 2):
                nc.tensor.matmul(
                    ps[:], wqk_t(fc, kc),
                    xt[kc][:, t4w * 512:(t4w + 1) * 512],
                    start=(kc == 0), stop=(kc == KC - 1))
            if quarter == 3:
                nc.vector.tensor_scalar_add(
                    qkt[fc][:, t4w * 512:(t4w + 1) * 512], ps[:],
                    bqk_sb[:, fc:fc + 1])

        def v_item(tokc, quarter):
            ps = dbank[0]
            for kc in range(quarter * 2, quarter * 2 + 2):
                nc.tensor.matmul(
                    ps[:], xt[kc][:, tokc * 128:(tokc + 1) * 128],
                    wv[kc], start=(kc == 0), stop=(kc == KC - 1))
            if quarter == 3:
                nc.vector.tensor_copy(
                    v2[tokc].rearrange("p (h c) -> p h c", c=65)[:, :, 0:64],
                    ps.rearrange("p (h c) -> p h c", c=64)[:])

        def transp_item(pair, t4w):
            tp = dbank[1].bitcast(BF16)
            yv = [ysb[pair][s].rearrange("p (t c) -> p t c", c=64)
                  for s in range(2)]
            first_mm = None
            mm = None
            for s in range(2):
                for j in range(NQ):
                    tc_idx = t4w * NQ + j
                    mm = nc.tensor.matmul(
                        tp[s * 64:(s + 1) * 64, j * 128:(j + 1) * 128],
                        yv[s][:, tc_idx, :], id_sb[:],
                        start=True, stop=True, is_transpose=True)
                    if j == 0:
                        # RAW on ysb: the normalize (DVE) must land first
                        _sync_after(mm, norm_mul[(pair, t4w, s)])
                    if first_mm is None:
                        first_mm = mm
                        if last_tp_copy[0] is not None:
                            # WAR: PE must not overwrite the bank before
                            # the previous item's copy has drained it
                            _sync_after(mm, last_tp_copy[0])
            cp = nc.vector.tensor_copy(
                yt[pair][:, t4w * 512:(t4w + 1) * 512], tp[:, 0:512])
            # RAW: the copy needs all 8 transposes (PE completes in order,
            # so syncing on the last one is sufficient)
            _sync_after(cp, mm)
            last_tp_copy[0] = cp

        def proj_item(occ, t4w):
            ps = dbank[0]
            for fcp in range(PCH):
                nc.tensor.matmul(
                    ps[:], wp[fcp][:, occ * 128:(occ + 1) * 128],
                    yt[fcp][:, t4w * 512:(t4w + 1) * 512],
                    start=(fcp == 0), stop=(fcp == PCH - 1))
            osb = osbp.tile([128, 512], BF16, tag="osb")
            nc.vector.tensor_scalar_add(osb[:], ps[:],
                                        bout_sb[:, occ:occ + 1])
            nc.sync.dma_start(
                out_t[occ * 128:(occ + 1) * 128,
                      t4w * 512:(t4w + 1) * 512], osb[:])

        # ---------- iteration stream ----------
        iters = [(pair, win, kc) for pair in range(NPAIR)
                 for win in range(NW) for kc in range(TC1)]
        NIT = len(iters)
        et_handles = {}

        def emit_scores_act(j):
            pair, win, kc = iters[j]
            reg = sreg[j % 2]
            q0 = win * QW
            qt, kt = qkt[pair], qkt[NPAIR + pair]
            for s in range(2):
                po = s * 64
                nc.tensor.matmul(
                    reg[:, s * 512:(s + 1) * 512],
                    kt[po:po + 64, kc * 128:(kc + 1) * 128],
                    qt[po:po + 64, q0:q0 + QW],
                    start=True, stop=True)
            et = ep.tile([128, 1024], BF16, tag="et", name=f"et_{j}")
            nc.scalar.activation(et[:], reg[:], Exp, scale=0.125)
            et_handles[j] = et

        def emit_av(j):
            pair, win, kc = iters[j]
            et = et_handles.pop(j)
            first, last = (kc == 0), (kc == TC1 - 1)
            # one start per ups bank per window: start zeroes the full 2KB
            # zero region.  The qi>0 kc==0 writes must execute after the
            # start MM; they share its deps (same et tile, same WAR on the
            # previous window's normalize), so the scheduler's priority
            # heap preserves emission order — no explicit edges needed
            # (explicit nosync edges here measured 10x slower on HW).
            for s in range(2):
                h = 2 * pair + s
                for qi in range(NQ):
                    mm = nc.tensor.matmul(
                        ups[s][:, qi * 65:qi * 65 + 65],
                        et[:, s * 512 + qi * 128:s * 512 + qi * 128 + 128],
                        v2[kc][:, h * 65:h * 65 + 65],
                        start=(first and qi == 0),
                        stop=(last and qi == NQ - 1))
                    if last and qi == NQ - 1:
                        av_last[s] = mm

        def emit_norm(pair, win):
            rden = recp.tile([128, 2 * NQ], F32, tag="rden",
                             name=f"rden_{pair}_{win}")
            for s in range(2):
                uv = ups[s][:, 0:NQ * 65].rearrange("p (q c) -> p q c", c=65)
                rc = nc.vector.reciprocal(
                    rden[:, s * NQ:(s + 1) * NQ].unsqueeze(2),
                    uv[:, :, 64:65])
                # rearranged-AP reads are not reliably connected to the
                # plain-AP AV writes: order explicitly on the stop matmul
                _sync_after(rc, av_last[s])
                yv = ysb[pair][s].rearrange("p (w q d) -> p w q d",
                                            w=NW, d=64)
                ml = nc.vector.tensor_tensor(
                    yv[:, win, :, :], uv[:, :, 0:64],
                    rden[:, s * NQ:(s + 1) * NQ].unsqueeze(2)
                    .to_broadcast((128, NQ, 64)),
                    op=Mult)
                norm_mul[(pair, win, s)] = ml

        # ---------- filler queue: (ready_iter, deadline_iter, fn) ----------
        # ready: don't pop before this iteration (the item's inputs exist).
        # deadline: MUST be emitted before this iteration's scores/AV (the
        # scheduler keeps per-engine emission order, so a consumer emitted
        # before its producer reads garbage).
        NIT = NPAIR * NW * TC1
        queue = deque()
        for tokc in range(2, TC1):
            for hf in range(4):
                # v2[tokc] is read by emit_av at iteration tokc
                queue.append((0, tokc, lambda tokc=tokc, hf=hf:
                              v_item(tokc, hf)))
        for p in range(1, NPAIR):
            for fc in (p, NPAIR + p):
                for t4w in range(NW):
                    for hf in range(4):
                        # qkt[fc] read by scores of pair p (emitted at
                        # iteration p*NW*TC1 - 1 via the j+1 lookahead)
                        queue.append(
                            (0, p * NW * TC1 - 1,
                             lambda fc=fc, t4w=t4w, hf=hf:
                             qk_item(fc, t4w, hf)))
        # transposes: (p, t4w) ready after iteration (p, win=t4w, kc last);
        # proj(occ, t4w) ready after the LAST pair's window t4w.
        # pops happen at iteration START.  With the L=2 av lag, window
        # (p, w) [ending at iteration e = (p*NW+w)*TC1 + TC1 - 1] has its
        # norm emitted at the END of slot e+1, so the earliest safe pop of
        # a consumer is the start of slot e+2, i.e. ready = e+3.
        tr_pr = []
        for t4w in range(NW):
            for p in range(NPAIR):
                rdy = (p * NW + t4w) * TC1 + TC1 + 2
                tr_pr.append((rdy, 0, lambda p=p, t4w=t4w:
                              transp_item(p, t4w)))
        for t4w in range(NW):
            rdy = ((NPAIR - 1) * NW + t4w) * TC1 + TC1 + 2
            for occ in range(OCC):
                tr_pr.append((rdy, 1, lambda occ=occ, t4w=t4w:
                              proj_item(occ, t4w)))
        tr_pr.sort(key=lambda x: (x[0], x[1]))
        for rdy, _, fn in tr_pr:
            queue.append((rdy, NIT + 2, fn))

        # ---------- upfront: QK(pair0) + first two V chunks ----------
        for fc in (0, NPAIR):
            for t4w in range(NW):
                for q in range(4):
                    qk_item(fc, t4w, q)
        for tokc in range(min(2, TC1)):
            for q in range(4):
                v_item(tokc, q)

        # ---------- main loop ----------
        # All filler pops are emitted BEFORE scores(j+1): the PE is gated
        # on ACT completions at scores (PSUM WAR) and at av (et read), so
        # work placed after those gates lands in the scores<->ACT critical
        # cycle and directly stretches the period.
        # av lags scores by 2 iterations (L=2): av(j-1)'s ACT finished a
        # full period ago, so the PE only ever parks at the scores WAR gate.
        emit_scores_act(0)
        for j in range(NIT + 1):
            # deadline items first (correctness), then one budgeted filler
            while queue and queue[0][1] <= j + 1:
                queue.popleft()[2]()
            if queue and queue[0][0] <= j + 1:
                queue.popleft()[2]()
            if j + 1 < NIT:
                emit_scores_act(j + 1)
            if 1 <= j <= NIT:
                emit_av(j - 1)
                pair, win, kc = iters[j - 1]
                if kc == TC1 - 1:
                    emit_norm(pair, win)
        while queue:
            queue.popleft()[2]()


def build_nc(T=2048):
    FC = 2 * CG // 128
    OCC = C // 128
    nc = bacc.Bacc("TRN2", target_bir_lowering=False, debug=False,
                   num_devices=N_CORES)
    x_t = nc.dram_tensor("x_t", [C, T], BF16, kind="ExternalInput")
    w_qk = nc.dram_tensor("w_qk", [C, 2 * CG], BF16, kind="ExternalInput")
    b_qk = nc.dram_tensor("b_qk", [128, FC], F32, kind="ExternalInput")
    w_v = nc.dram_tensor("w_v", [C, CG], BF16, kind="ExternalInput")
    w_p = nc.dram_tensor("w_p", [CG, C], BF16, kind="ExternalInput")
    b_out = nc.dram_tensor("b_out", [128, OCC], F32, kind="ExternalInput")
    id_bf = nc.dram_tensor("id_bf", [128, 128], BF16, kind="ExternalInput")
    out_t = nc.dram_tensor("out_t", [C, T], BF16, kind="ExternalOutput")
    with tile.TileContext(nc) as tc:
        _body(tc, T, x_t.ap(), w_qk.ap(), b_qk.ap(), w_v.ap(),
              w_p.ap(), b_out.ap(), id_bf.ap(), out_t.ap())
    nc.compile()
    return nc


def shard_inputs(sequences, w_attn, b_attn, w_proj, b_proj):
    """Build the 8 per-core input maps. Core index = b*2 + g."""
    sequences = np.asarray(sequences, dtype=np.float32)
    w_attn = np.asarray(w_attn, dtype=np.float32)
    b_attn = np.asarray(b_attn, dtype=np.float32)
    w_proj = np.asarray(w_proj, dtype=np.float32)
    b_proj = np.asarray(b_proj, dtype=np.float32)
    B = sequences.shape[0]
    ident = np.eye(128, dtype=ml_dtypes.bfloat16)
    in_maps = []
    for b in range(B):
        for g in range(2):
            qs = slice(g * CG, (g + 1) * CG)
            ks = slice(C + g * CG, C + (g + 1) * CG)
            vs = slice(2 * C + g * CG, 2 * C + (g + 1) * CG)
            in_maps.append({
                "x_t": np.ascontiguousarray(sequences[b].T)
                    .astype(ml_dtypes.bfloat16),
                "w_qk": np.ascontiguousarray(
                    np.concatenate([w_attn[:, qs], w_attn[:, ks]], axis=1))
                    .astype(ml_dtypes.bfloat16),
                "b_qk": np.ascontiguousarray(
                    np.concatenate([b_attn[qs], b_attn[ks]])
                    .reshape(8, 128).T),
                "w_v": np.ascontiguousarray(w_attn[:, vs])
                    .astype(ml_dtypes.bfloat16),
                "w_p": np.ascontiguousarray(w_proj[g * CG:(g + 1) * CG, :])
                    .astype(ml_dtypes.bfloat16),
                # softmax rows sum to 1, so the v-bias folds into the output
                # bias: y_g = attn@(x@w_v) @ w_p + (b_v@w_p [+ b_proj on g0])
                "b_out": np.ascontiguousarray(
                    (b_attn[vs] @ w_proj[g * CG:(g + 1) * CG, :]
                     + (b_proj if g == 0 else 0.0))
                    .astype(np.float32).reshape(8, 128).T),
                "id_bf": ident,
            })
    return in_maps


def unshard_outputs(outs, B, T):
    """outs: list of 8 [C, T] partials, core index = b*2+g."""
    y = np.empty((B, T, C), np.float32)
    for b in range(B):
        y[b] = (np.asarray(outs[2 * b], np.float32)
                + np.asarray(outs[2 * b + 1], np.float32)).T
    return y


_NC_CACHE = {}


def kernel(sequences, w_attn, b_attn, w_proj, b_proj):
    sequences = np.asarray(sequences, dtype=np.float32)
    B, T, _ = sequences.shape
    in_maps = shard_inputs(sequences, w_attn, b_attn, w_proj, b_proj)
    if T not in _NC_CACHE:
        _NC_CACHE[T] = build_nc(T)
    nc = _NC_CACHE[T]
    res = run_bass_kernel_spmd(nc, in_maps, list(range(N_CORES)))
    outs = [res.results[i]["out_t"] for i in range(N_CORES)]
    return unshard_outputs(outs, B, T)


if __name__ == "__main__":
    rng = np.random.default_rng(0)
    B, T = 4, 2048
    seq = rng.standard_normal((B, T, C), dtype=np.float32)
    wa = rng.standard_normal((C, 3 * C), dtype=np.float32) / np.sqrt(C)
    ba = np.zeros(3 * C, np.float32)
    wpj = rng.standard_normal((C, C), dtype=np.float32) / np.sqrt(C)
    bp = np.zeros(C, np.float32)
    y = kernel(seq, wa, ba, wpj, bp)
    print(y.shape, y.dtype)
